# revision 9
# baseline (speedup 1.0000x reference)
"""Expert-parallel sparse MoE kernel for Trainium2 (8 NeuronCores).

Reference model: dense MoE (every expert on every token) followed by a
top-2-sparse combine, residual add, and LayerNorm.  Mathematically only the
top-2 experts per token contribute to the output, so the kernel routes each
token to its top-2 experts and only computes those expert FFNs.

Sharding: expert-parallel.  Each of the 8 cores owns 8 of the 64 experts and
receives the tokens routed to them (all-to-all by routing, done host-side as
part of sharding).  The device streams the expert weights (the dominant
memory traffic, fp8) into a fully SBUF-resident buffer -- all weight DMAs are
issued up-front with no buffer recycling, so the HBM stream runs gap-free at
line rate.  Compute chases the stream one expert at a time:
  mm1: h^T = relu(W1^T x^T + b1)   (w1 stationary fp8+FWL, x moving, N=96)
  mm2: y = h^T.T @ W2              (DoubleRow fp8: both operands fp8,
                                    256-row contraction, 8 MMs of N=512)
The host applies gate weights + b2 during the unshard/scatter, adds the
residual, and normalizes.
"""

import numpy as np
import ml_dtypes

B, S, D, H, E, TOPK = 2, 1024, 512, 2048, 64, 2
T = B * S
NCORES = 8
EPC = E // NCORES          # experts per core
CAP = 96                   # token capacity per expert (observed max 95;
                           # overflow tokens fall back to exact host compute)
DC = D // 128              # 4 contraction chunks for x @ W1
HC = H // 128              # 16 contraction chunks for h @ W2
EPS = 1e-5
BF16 = ml_dtypes.bfloat16
NWARM = 9                  # dummy N=512 matmuls that hold the PE HAM
                           # clock-gate open while the first weight DMA
                           # is in flight (~430ns each cold)

PROFILE = False            # set True (module-level) to capture an NTFF trace
LAST_RESULT = None         # BassKernelResults of the last run (for test.py)

# fp8 everywhere on-device: W1/W2 scaled by WSCALE host-side and stored e4m3;
# x and the hidden h are also e4m3 (their error is ~2-3% relative on the
# expert outputs, which enter the final LN-dominated output at ~0.03x -- far
# inside the 2e-2 absmax tolerance).  1/WSCALE^2 descale folds into the
# host-side combine.
WSCALE = 16.0
FP8 = ml_dtypes.float8_e4m3fn

_NC_CACHE = {}


def _build_bass():
    """Build the per-core Bass/Tile program (identical on all 8 cores)."""
    import concourse.bacc as bacc
    import concourse.mybir as mybir
    from concourse import tile

    nc = bacc.Bacc("TRN2", target_bir_lowering=False, debug=False,
                   num_devices=NCORES)

    bf = mybir.dt.bfloat16
    f32 = mybir.dt.float32
    f8 = mybir.dt.float8e4
    xt = nc.dram_tensor("xt", [128, EPC, DC, CAP], f8, kind="ExternalInput")
    # W1|W2 fused per expert: [d-part, DC*H (w1) + HC*D (w2)]
    w12 = nc.dram_tensor("w12", [EPC, 128, DC * H + HC * D], f8,
                         kind="ExternalInput")
    b1 = nc.dram_tensor("b1", [128, EPC, HC], f32, kind="ExternalInput")
    y = nc.dram_tensor("y", [EPC, CAP, D], bf, kind="ExternalOutput")

    relu = mybir.ActivationFunctionType.Relu
    DR = mybir.MatmulPerfMode.DoubleRow
    alu_add = mybir.AluOpType.add
    alu_max = mybir.AluOpType.max

    # w1 per-expert DRAM layout: [128, 2, DC, H/2] -- H-halves outermost so a
    # half-w1 DMA is per-partition contiguous (4 KiB descriptors; 1 KiB
    # strided descriptors measured ~2.5x slower on the HWDGE ring).
    src_w1 = lambda i: w12[i][:, :DC * H].rearrange(
        "p (s c h) -> p s c h", s=2, c=DC)
    src_w2 = lambda i: w12[i][:, DC * H:].rearrange("p (c d) -> p c d", c=HC)

    with tile.TileContext(nc) as tc:
        with (
            tc.tile_pool(name="big", bufs=1) as big,
            tc.tile_pool(name="acts", bufs=2) as acts,
            tc.tile_pool(name="ps1", bufs=4, space="PSUM") as ps1,
            tc.tile_pool(name="ps2", bufs=2, space="PSUM") as ps2,
            tc.tile_pool(name="psd", bufs=1, space="PSUM") as psd,
        ):
            # SBUF-resident weights for all 8 experts (128 KiB/partition).
            w1r = big.tile([128, EPC, 2, DC, H // 2], f8, name="w1r")
            w2r = big.tile([128, EPC, HC, D], f8, name="w2r")
            xtt = big.tile([128, EPC, DC, CAP], f8, name="xtt")
            b1t = big.tile([128, EPC, HC], f32, name="b1t")
            warm = big.tile([128, 512], f8, name="warm")
            warmf = big.tile([128, 1], f32, name="warmf")
            scr = big.tile([128, 1], f32, name="scr")

            # ---- DMA issue plan (per-engine queues drain in program order)
            # sync/HWDGE ring 1: expert-0 weights first (the DMA fabric is
            # nearly dead before ~10us and HWDGE starts ~3us before SWDGE),
            # w1 in quarters so mm1 can start on the first 0.25 MB; later
            # the y stores.
            for q in range(4):
                hh, hq = divmod(q, 2)
                sl = slice(hq * (H // 4), (hq + 1) * (H // 4))
                nc.sync.dma_start(w1r[:, 0, hh, :, sl], src_w1(0)[:, hh, :, sl])
            nc.sync.dma_start(w2r[:, 0, :HC // 2], src_w2(0)[:, :HC // 2])
            nc.sync.dma_start(w2r[:, 0, HC // 2:], src_w2(0)[:, HC // 2:])
            # scalar/HWDGE ring 2: biases first (tiny, unblocks the relu
            # path), then tokens for experts 0-1, then the rest.
            nc.scalar.dma_start(b1t[:], b1[:])
            nc.scalar.dma_start(xtt[:, :2], xt[:, :2])
            nc.scalar.dma_start(xtt[:, 2:], xt[:, 2:])
            # gpsimd/SWDGE: experts 1..7 (the bulk, ~14 MB), in consumption
            # order.  The last expert is split finer so its compute can
            # chase the final bytes of the stream.
            for i in range(1, EPC):
                if i == EPC - 1:
                    nc.gpsimd.dma_start(w1r[:, i, 0], src_w1(i)[:, 0])
                    nc.gpsimd.dma_start(w1r[:, i, 1], src_w1(i)[:, 1])
                    nc.gpsimd.dma_start(w2r[:, i, :HC // 2],
                                        src_w2(i)[:, :HC // 2])
                    nc.gpsimd.dma_start(w2r[:, i, HC // 2:],
                                        src_w2(i)[:, HC // 2:])
                else:
                    nc.gpsimd.dma_start(w1r[:, i], src_w1(i))
                    nc.gpsimd.dma_start(w2r[:, i], src_w2(i))

            # ---- PE pre-warm: dummy matmuls on a zeroed tile keep the PE
            # busy from ~7us so the HAM clock-gate opens (2.4 GHz) right as
            # the first real matmul becomes ready (~11us).  They must run
            # back-to-back into the real stream -- any >1us PE idle resets
            # the HAM busy-window accumulation.
            nc.vector.memset(warm[:], 0.0)
            nc.vector.memset(warmf[:], 0.0)
            pd = psd.tile([64, 512], f32, name="pd")
            for _ in range(NWARM):
                nc.tensor.matmul(pd[:], warm[:, :64], warm[:],
                                 start=True, stop=True)
            # ACT-table preload (first activation pays a ~1.3us table load).
            # Reads only the locally-memset warmf -- it must NOT wait on any
            # DMA, or it blocks the whole relu queue behind it.
            nc.scalar.activation(scr[:], warmf[:], relu, bias=warmf[:])

            # ---- per-expert compute, chasing the weight stream
            for i in range(EPC):
                # h^T = relu(W1^T x^T + b1), produced [h, token] so mm2 can
                # contract over h on the partition dim.  The relu+bias (which
                # also casts to fp8 for DoubleRow mm2) alternates between
                # ScalarE and VectorE to halve the serial cost.
                ht = acts.tile([128, HC, CAP], f8, name="ht")
                for j in range(HC):
                    p1 = ps1.tile([128, CAP], f32, name="p1")
                    hh, jj = divmod(j, HC // 2)
                    for c in range(DC):
                        nc.tensor.matmul(
                            p1[:],
                            w1r[:, i, hh, c, jj * 128:(jj + 1) * 128],
                            xtt[:, i, c, :],
                            start=(c == 0),
                            stop=(c == DC - 1),
                        )
                    if j % 2 == 0:
                        nc.scalar.activation(ht[:, j, :], p1[:], relu,
                                             bias=b1t[:, i, j:j + 1])
                    else:
                        nc.vector.tensor_scalar(
                            ht[:, j, :], p1[:], b1t[:, i, j:j + 1], 0.0,
                            alu_add, alu_max)
                # mm2 in DoubleRow: both operands fp8, 256-row contraction
                # per MM -> 8 matmuls of N=512 instead of 16.
                p2 = ps2.tile([CAP, D], f32, name="p2")
                for k in range(HC // 2):
                    nc.tensor.matmul(
                        p2[:],
                        ht[:, 2 * k:2 * k + 2, :],
                        w2r[:, i, 2 * k:2 * k + 2, :],
                        start=(k == 0),
                        stop=(k == HC // 2 - 1),
                        perf_mode=DR,
                    )
                yt = acts.tile([CAP, D], bf, name="yt")
                nc.vector.tensor_copy(yt[:], p2[:])
                nc.sync.dma_start(y[i], yt[:])

    # Bacc lowering: splits excess per-instruction sem waits onto
    # InstEventSemaphore, moves matmul waits onto ldweights, inserts
    # activation table loads -- required for walrus codegen (1 wait slot
    # per 64B ISA instruction).
    nc.compile()
    return nc


def _get_nc():
    if "nc" not in _NC_CACHE:
        _NC_CACHE["nc"] = _build_bass()
    return _NC_CACHE["nc"]


def kernel(x, Wg, bg, W1, b1, W2, b2, gamma, beta):
    global LAST_RESULT
    x = np.asarray(x, np.float32)
    Wg = np.asarray(Wg, np.float32)
    bg = np.asarray(bg, np.float32)
    W1 = np.asarray(W1, np.float32)
    b1 = np.asarray(b1, np.float32)
    W2 = np.asarray(W2, np.float32)
    b2 = np.asarray(b2, np.float32)
    gamma = np.asarray(gamma, np.float32)
    beta = np.asarray(beta, np.float32)

    xf = x.reshape(T, D)

    # ---- gating: softmax over experts, top-2 (ties -> lower index, as top_k)
    logits = xf @ Wg + bg
    logits -= logits.max(-1, keepdims=True)
    probs = np.exp(logits)
    probs /= probs.sum(-1, keepdims=True)
    idx = np.argsort(-probs, axis=-1, kind="stable")[:, :TOPK]   # [T, K]
    vals = np.take_along_axis(probs, idx, axis=-1)               # [T, K]

    # ---- per-expert token lists (the all-to-all "sharding by routing")
    slot = np.full((T, TOPK), -1, np.int64)
    toks_per_e = []
    overflow = []  # (expert, token_ids) pairs beyond CAP -> host fallback
    for e in range(E):
        te = np.nonzero((idx == e).any(-1))[0]
        if len(te) > CAP:
            overflow.append((e, te[CAP:]))
            te = te[:CAP]
        toks_per_e.append(te)
        if len(te):
            k_of = (idx[te] == e).argmax(-1)
            slot[te, k_of] = np.arange(len(te))

    # ---- pack per-core device inputs (layouts match SBUF tiles exactly)
    xth = np.zeros((E, 128, DC, CAP), FP8)
    for e in range(E):
        te = toks_per_e[e]
        if len(te):
            blk = xf[te].T.reshape(DC, 128, len(te)).transpose(1, 0, 2)
            xth[e, :, :, :len(te)] = blk.astype(FP8)
    wq = lambda a: (a * WSCALE).astype(FP8)
    # [e, p, s, c, h']: H-halves (s) outermost, then the DC contraction
    # chunks -- matches the w1r SBUF tile so half-w1 DMAs are contiguous.
    w1h = wq(W1).reshape(E, DC, 128, 2, H // 2).transpose(0, 2, 3, 1, 4)
    w2h = wq(W2).reshape(E, HC, 128, D).transpose(0, 2, 1, 3)
    w12h = np.concatenate([w1h.reshape(E, 128, DC * H),
                           w2h.reshape(E, 128, HC * D)], axis=2)
    b1s = b1 * WSCALE
    b1h = np.ascontiguousarray(b1s.reshape(E, HC, 128).transpose(0, 2, 1))

    in_maps = []
    for c in range(NCORES):
        sl = slice(c * EPC, (c + 1) * EPC)
        in_maps.append({
            "xt": np.ascontiguousarray(xth[sl].transpose(1, 0, 2, 3)),
            "w12": w12h[sl],
            "b1": np.ascontiguousarray(b1h[sl].transpose(1, 0, 2)),
        })

    # ---- run on the 8 cores
    from concourse.bass_utils import run_bass_kernel_spmd
    nc = _get_nc()
    res = run_bass_kernel_spmd(nc, in_maps, list(range(NCORES)),
                               trace=PROFILE)
    LAST_RESULT = res
    y_all = np.concatenate([r["y"] for r in res.results],
                           axis=0).astype(np.float32)             # [E,CAP,D]
    y_all /= WSCALE * WSCALE

    # ---- unshard: scatter expert outputs back by routing, combine, LN
    ok = slot >= 0
    sl = np.where(ok, slot, 0)
    contrib = y_all[idx, sl] + b2[idx]                 # [T, K, D]
    out = xf + (vals[..., None] * contrib * ok[..., None]).sum(1)

    for e, te in overflow:  # practically never taken (CAP >> max count)
        k_of = (idx[te] == e).argmax(-1)
        w = vals[te, k_of]
        h = np.maximum(xf[te] @ W1[e] + b1[e], 0.0)
        out[te] += w[:, None] * (h @ W2[e] + b2[e])

    mu = out.mean(-1, keepdims=True)
    var = ((out - mu) ** 2).mean(-1, keepdims=True)
    o = (out - mu) / np.sqrt(var + EPS) * gamma + beta
    return o.reshape(B, S, D).astype(np.float32)


# revision 15
# speedup vs baseline: 1.1580x; 1.1580x over previous
"""Expert-parallel sparse MoE kernel for Trainium2 (8 NeuronCores).

Reference model: dense MoE (every expert on every token) followed by a
top-2-sparse combine, residual add, and LayerNorm.  Mathematically only the
top-2 experts per token contribute to the output, so the kernel routes each
token to its top-2 experts and only computes those expert FFNs.

Sharding: expert-parallel.  Each of the 8 cores owns 8 of the 64 experts and
receives the tokens routed to them (all-to-all by routing, done host-side as
part of sharding).  The device streams the expert weights (the dominant
memory traffic, fp8) into a fully SBUF-resident buffer -- all weight DMAs are
issued up-front with no buffer recycling, so the HBM stream runs gap-free at
line rate.  Compute chases the stream one expert at a time:
  mm1: h^T = relu(W1^T x^T + b1)   (w1 stationary fp8+FWL, x moving, N=96)
  mm2: y = h^T.T @ W2              (DoubleRow fp8: both operands fp8,
                                    256-row contraction, 8 MMs of N=512)
The host applies gate weights + b2 during the unshard/scatter, adds the
residual, and normalizes.
"""

import numpy as np
import ml_dtypes

B, S, D, H, E, TOPK = 2, 1024, 512, 2048, 64, 2
T = B * S
NCORES = 8
EPC = E // NCORES          # experts per core
CAP = 96                   # token capacity per expert (observed max 95;
                           # overflow tokens fall back to exact host compute)
DC = D // 128              # 4 contraction chunks for x @ W1
HC = H // 128              # 16 contraction chunks for h @ W2
EPS = 1e-5
BF16 = ml_dtypes.bfloat16
NWARM = 9                  # dummy N=512 matmuls that hold the PE HAM
                           # clock-gate open while the first weight DMA
                           # is in flight (~430ns each cold)

PROFILE = False            # set True (module-level) to capture an NTFF trace
LAST_RESULT = None         # BassKernelResults of the last run (for test.py)

# fp8 everywhere on-device: W1/W2 scaled by WSCALE host-side and stored e4m3;
# x and the hidden h are also e4m3 (their error is ~2-3% relative on the
# expert outputs, which enter the final LN-dominated output at ~0.03x -- far
# inside the 2e-2 absmax tolerance).  1/WSCALE^2 descale folds into the
# host-side combine.
WSCALE = 16.0
FP8 = ml_dtypes.float8_e4m3fn

_NC_CACHE = {}


def _build_bass():
    """Build the per-core Bass/Tile program (identical on all 8 cores)."""
    import concourse.bacc as bacc
    import concourse.mybir as mybir
    from concourse import tile

    nc = bacc.Bacc("TRN2", target_bir_lowering=False, debug=False,
                   num_devices=NCORES)

    bf = mybir.dt.bfloat16
    f32 = mybir.dt.float32
    f8 = mybir.dt.float8e4
    xt = nc.dram_tensor("xt", [128, EPC, DC, CAP], f8, kind="ExternalInput")
    # W1|W2 fused per expert: [d-part, DC*H (w1) + HC*D (w2)]
    w12 = nc.dram_tensor("w12", [EPC, 128, DC * H + HC * D], f8,
                         kind="ExternalInput")
    b1 = nc.dram_tensor("b1", [128, EPC, HC], f32, kind="ExternalInput")
    y = nc.dram_tensor("y", [EPC, CAP, D], bf, kind="ExternalOutput")

    relu = mybir.ActivationFunctionType.Relu
    DR = mybir.MatmulPerfMode.DoubleRow
    alu_add = mybir.AluOpType.add
    alu_max = mybir.AluOpType.max

    # w1 per-expert DRAM layout: [128, 4, DC, H/4] -- H-quarters outermost so
    # a quarter-w1 DMA is per-partition contiguous (2 KiB descriptors; sub-1
    # KiB strided descriptors measured 3-8x slower on the HWDGE ring).
    src_w1 = lambda i: w12[i][:, :DC * H].rearrange(
        "p (s c h) -> p s c h", s=4, c=DC)
    src_w2 = lambda i: w12[i][:, DC * H:].rearrange("p (c d) -> p c d", c=HC)

    with tile.TileContext(nc) as tc:
        with (
            tc.tile_pool(name="big", bufs=1) as big,
            tc.tile_pool(name="acts", bufs=2) as acts,
            tc.tile_pool(name="ps1", bufs=4, space="PSUM") as ps1,
            tc.tile_pool(name="ps2", bufs=2, space="PSUM") as ps2,
            tc.tile_pool(name="psd", bufs=1, space="PSUM") as psd,
        ):
            # SBUF-resident weights for all 8 experts (128 KiB/partition).
            w1r = big.tile([128, EPC, 4, DC, H // 4], f8, name="w1r")
            w2r = big.tile([128, EPC, HC, D], f8, name="w2r")
            xtt = big.tile([128, EPC, DC, CAP], f8, name="xtt")
            b1t = big.tile([128, EPC, HC], f32, name="b1t")
            warm = big.tile([128, 512], f8, name="warm")
            warmf = big.tile([128, 1], f32, name="warmf")
            scr = big.tile([128, 1], f32, name="scr")

            # ---- DMA issue plan (per-engine queues drain in program order)
            # sync/HWDGE ring 1: expert-0 weights first (the DMA fabric is
            # nearly dead before ~10us and HWDGE starts ~3us before SWDGE),
            # w1 in quarters so mm1 can start on the first 0.25 MB; later
            # the y stores.
            for q in range(4):
                nc.sync.dma_start(w1r[:, 0, q], src_w1(0)[:, q])
            nc.sync.dma_start(w2r[:, 0, :HC // 2], src_w2(0)[:, :HC // 2])
            nc.sync.dma_start(w2r[:, 0, HC // 2:], src_w2(0)[:, HC // 2:])
            # scalar/HWDGE ring 2: biases first (tiny, unblocks the relu
            # path), then tokens for experts 0-1, then the rest.
            nc.scalar.dma_start(b1t[:], b1[:])
            nc.scalar.dma_start(xtt[:, :2], xt[:, :2])
            nc.scalar.dma_start(xtt[:, 2:], xt[:, 2:])
            # gpsimd/SWDGE: experts 1..7 (the bulk, ~14 MB), in consumption
            # order.  The last expert is split finer so its compute can
            # chase the final bytes of the stream.
            for i in range(1, EPC):
                if i == EPC - 1:
                    nc.gpsimd.dma_start(w1r[:, i, :2], src_w1(i)[:, :2])
                    nc.gpsimd.dma_start(w1r[:, i, 2:], src_w1(i)[:, 2:])
                    nc.gpsimd.dma_start(w2r[:, i, :HC // 2],
                                        src_w2(i)[:, :HC // 2])
                    nc.gpsimd.dma_start(w2r[:, i, HC // 2:],
                                        src_w2(i)[:, HC // 2:])
                else:
                    nc.gpsimd.dma_start(w1r[:, i], src_w1(i))
                    nc.gpsimd.dma_start(w2r[:, i], src_w2(i))

            # ---- PE pre-warm: dummy matmuls on a zeroed tile keep the PE
            # busy from ~7us so the HAM clock-gate opens (2.4 GHz) right as
            # the first real matmul becomes ready (~11us).  They must run
            # back-to-back into the real stream -- any >1us PE idle resets
            # the HAM busy-window accumulation.
            nc.vector.memset(warm[:], 0.0)
            nc.vector.memset(warmf[:], 0.0)
            pd = psd.tile([64, 512], f32, name="pd")
            for _ in range(NWARM):
                nc.tensor.matmul(pd[:], warm[:, :64], warm[:],
                                 start=True, stop=True)
            # ACT-table preload (first activation pays a ~1.3us table load).
            # Reads only the locally-memset warmf -- it must NOT wait on any
            # DMA, or it blocks the whole relu queue behind it.
            nc.scalar.activation(scr[:], warmf[:], relu, bias=warmf[:])

            # ---- per-expert compute, chasing the weight stream
            for i in range(EPC):
                # h^T = relu(W1^T x^T + b1), produced [h, token] so mm2 can
                # contract over h on the partition dim.  The relu+bias (which
                # also casts to fp8 for DoubleRow mm2) alternates between
                # ScalarE and VectorE to halve the serial cost.
                ht = acts.tile([128, HC, CAP], f8, name="ht")
                for j in range(HC):
                    p1 = ps1.tile([128, CAP], f32, name="p1")
                    hh, jj = divmod(j, HC // 4)
                    for c in range(DC):
                        nc.tensor.matmul(
                            p1[:],
                            w1r[:, i, hh, c, jj * 128:(jj + 1) * 128],
                            xtt[:, i, c, :],
                            start=(c == 0),
                            stop=(c == DC - 1),
                        )
                    if j % 2 == 0:
                        nc.scalar.activation(ht[:, j, :], p1[:], relu,
                                             bias=b1t[:, i, j:j + 1])
                    else:
                        nc.vector.tensor_scalar(
                            ht[:, j, :], p1[:], b1t[:, i, j:j + 1], 0.0,
                            alu_add, alu_max)
                # mm2 in DoubleRow: both operands fp8, 256-row contraction
                # per MM -> 8 matmuls of N=512 instead of 16.
                p2 = ps2.tile([CAP, D], f32, name="p2")
                for k in range(HC // 2):
                    nc.tensor.matmul(
                        p2[:],
                        ht[:, 2 * k:2 * k + 2, :],
                        w2r[:, i, 2 * k:2 * k + 2, :],
                        start=(k == 0),
                        stop=(k == HC // 2 - 1),
                        perf_mode=DR,
                    )
                yt = acts.tile([CAP, D], bf, name="yt")
                nc.vector.tensor_copy(yt[:], p2[:])
                nc.sync.dma_start(y[i], yt[:])

    # Bacc lowering: splits excess per-instruction sem waits onto
    # InstEventSemaphore, moves matmul waits onto ldweights, inserts
    # activation table loads -- required for walrus codegen (1 wait slot
    # per 64B ISA instruction).
    nc.compile()
    return nc


def _get_nc():
    if "nc" not in _NC_CACHE:
        _NC_CACHE["nc"] = _build_bass()
    return _NC_CACHE["nc"]


def kernel(x, Wg, bg, W1, b1, W2, b2, gamma, beta):
    global LAST_RESULT
    x = np.asarray(x, np.float32)
    Wg = np.asarray(Wg, np.float32)
    bg = np.asarray(bg, np.float32)
    W1 = np.asarray(W1, np.float32)
    b1 = np.asarray(b1, np.float32)
    W2 = np.asarray(W2, np.float32)
    b2 = np.asarray(b2, np.float32)
    gamma = np.asarray(gamma, np.float32)
    beta = np.asarray(beta, np.float32)

    xf = x.reshape(T, D)

    # ---- gating: softmax over experts, top-2 (ties -> lower index, as top_k)
    logits = xf @ Wg + bg
    logits -= logits.max(-1, keepdims=True)
    probs = np.exp(logits)
    probs /= probs.sum(-1, keepdims=True)
    idx = np.argsort(-probs, axis=-1, kind="stable")[:, :TOPK]   # [T, K]
    vals = np.take_along_axis(probs, idx, axis=-1)               # [T, K]

    # ---- per-expert token lists (the all-to-all "sharding by routing")
    slot = np.full((T, TOPK), -1, np.int64)
    toks_per_e = []
    overflow = []  # (expert, token_ids) pairs beyond CAP -> host fallback
    for e in range(E):
        te = np.nonzero((idx == e).any(-1))[0]
        if len(te) > CAP:
            overflow.append((e, te[CAP:]))
            te = te[:CAP]
        toks_per_e.append(te)
        if len(te):
            k_of = (idx[te] == e).argmax(-1)
            slot[te, k_of] = np.arange(len(te))

    # ---- pack per-core device inputs (layouts match SBUF tiles exactly)
    xth = np.zeros((E, 128, DC, CAP), FP8)
    for e in range(E):
        te = toks_per_e[e]
        if len(te):
            blk = xf[te].T.reshape(DC, 128, len(te)).transpose(1, 0, 2)
            xth[e, :, :, :len(te)] = blk.astype(FP8)
    wq = lambda a: (a * WSCALE).astype(FP8)
    # [e, p, s, c, h']: H-quarters (s) outermost, then the DC contraction
    # chunks -- matches the w1r SBUF tile so piece-w1 DMAs are contiguous.
    w1h = wq(W1).reshape(E, DC, 128, 4, H // 4).transpose(0, 2, 3, 1, 4)
    w2h = wq(W2).reshape(E, HC, 128, D).transpose(0, 2, 1, 3)
    w12h = np.concatenate([w1h.reshape(E, 128, DC * H),
                           w2h.reshape(E, 128, HC * D)], axis=2)
    b1s = b1 * WSCALE
    b1h = np.ascontiguousarray(b1s.reshape(E, HC, 128).transpose(0, 2, 1))

    in_maps = []
    for c in range(NCORES):
        sl = slice(c * EPC, (c + 1) * EPC)
        in_maps.append({
            "xt": np.ascontiguousarray(xth[sl].transpose(1, 0, 2, 3)),
            "w12": w12h[sl],
            "b1": np.ascontiguousarray(b1h[sl].transpose(1, 0, 2)),
        })

    # ---- run on the 8 cores
    from concourse.bass_utils import run_bass_kernel_spmd
    nc = _get_nc()
    res = run_bass_kernel_spmd(nc, in_maps, list(range(NCORES)),
                               trace=PROFILE)
    LAST_RESULT = res
    y_all = np.concatenate([r["y"] for r in res.results],
                           axis=0).astype(np.float32)             # [E,CAP,D]
    y_all /= WSCALE * WSCALE

    # ---- unshard: scatter expert outputs back by routing, combine, LN
    ok = slot >= 0
    sl = np.where(ok, slot, 0)
    contrib = y_all[idx, sl] + b2[idx]                 # [T, K, D]
    out = xf + (vals[..., None] * contrib * ok[..., None]).sum(1)

    for e, te in overflow:  # practically never taken (CAP >> max count)
        k_of = (idx[te] == e).argmax(-1)
        w = vals[te, k_of]
        h = np.maximum(xf[te] @ W1[e] + b1[e], 0.0)
        out[te] += w[:, None] * (h @ W2[e] + b2[e])

    mu = out.mean(-1, keepdims=True)
    var = ((out - mu) ** 2).mean(-1, keepdims=True)
    o = (out - mu) / np.sqrt(var + EPS) * gamma + beta
    return o.reshape(B, S, D).astype(np.float32)


# revision 16
# speedup vs baseline: 1.4014x; 1.2102x over previous
"""Expert-parallel sparse MoE kernel for Trainium2 (8 NeuronCores).

Reference model: dense MoE (every expert on every token) followed by a
top-2-sparse combine, residual add, and LayerNorm.  Mathematically only the
top-2 experts per token contribute to the output, so the kernel routes each
token to its top-2 experts and only computes those expert FFNs.

Sharding: expert-parallel.  Each of the 8 cores owns 8 of the 64 experts and
receives the tokens routed to them (all-to-all by routing, done host-side as
part of sharding).  The device streams the expert weights (the dominant
memory traffic, fp8) into a fully SBUF-resident buffer -- all weight DMAs are
issued up-front with no buffer recycling, so the HBM stream runs gap-free at
line rate.  Compute chases the stream one expert at a time:
  mm1: h^T = relu(W1^T x^T + b1)   (w1 stationary fp8+FWL, x moving, N=96)
  mm2: y = h^T.T @ W2              (DoubleRow fp8: both operands fp8,
                                    256-row contraction, 8 MMs of N=512)
The host applies gate weights + b2 during the unshard/scatter, adds the
residual, and normalizes.
"""

import numpy as np
import ml_dtypes

B, S, D, H, E, TOPK = 2, 1024, 512, 2048, 64, 2
T = B * S
NCORES = 8
EPC = E // NCORES          # experts per core
CAP = 96                   # token capacity per expert (observed max 95;
                           # overflow tokens fall back to exact host compute)
DC = D // 128              # 4 contraction chunks for x @ W1
HC = H // 128              # 16 contraction chunks for h @ W2
EPS = 1e-5
BF16 = ml_dtypes.bfloat16
NWARM = 12                 # dummy N=512 matmuls that hold the PE HAM
                           # clock-gate open while the first weight DMA
                           # is in flight (~430ns each cold)

PROFILE = False            # set True (module-level) to capture an NTFF trace
LAST_RESULT = None         # BassKernelResults of the last run (for test.py)

# fp8 everywhere on-device: W1/W2 scaled by WSCALE host-side and stored e4m3;
# x and the hidden h are also e4m3 (their error is ~2-3% relative on the
# expert outputs, which enter the final LN-dominated output at ~0.03x -- far
# inside the 2e-2 absmax tolerance).  1/WSCALE^2 descale folds into the
# host-side combine.
WSCALE = 16.0
FP8 = ml_dtypes.float8_e4m3fn

_NC_CACHE = {}


def _build_bass():
    """Build the per-core Bass/Tile program (identical on all 8 cores)."""
    import concourse.bacc as bacc
    import concourse.mybir as mybir
    from concourse import tile

    nc = bacc.Bacc("TRN2", target_bir_lowering=False, debug=False,
                   num_devices=NCORES)

    bf = mybir.dt.bfloat16
    f32 = mybir.dt.float32
    f8 = mybir.dt.float8e4
    xt = nc.dram_tensor("xt", [128, EPC, DC, CAP], f8, kind="ExternalInput")
    # W1|W2 fused per expert: [d-part, DC*H (w1) + HC*D (w2)]
    w12 = nc.dram_tensor("w12", [EPC, 128, DC * H + HC * D], f8,
                         kind="ExternalInput")
    b1 = nc.dram_tensor("b1", [128, EPC, HC], f32, kind="ExternalInput")
    y = nc.dram_tensor("y", [EPC, CAP, D], bf, kind="ExternalOutput")

    relu = mybir.ActivationFunctionType.Relu
    DR = mybir.MatmulPerfMode.DoubleRow
    alu_add = mybir.AluOpType.add
    alu_max = mybir.AluOpType.max

    # w1 per-expert DRAM layout: [128, 2, DC, H/2] -- H-halves outermost so
    # a half-w1 DMA is per-partition contiguous (4 KiB descriptors; smaller
    # strided descriptors measured 2-8x slower on the HWDGE ring).
    src_w1 = lambda i: w12[i][:, :DC * H].rearrange(
        "p (s c h) -> p s c h", s=2, c=DC)
    src_w2 = lambda i: w12[i][:, DC * H:].rearrange("p (c d) -> p c d", c=HC)

    with tile.TileContext(nc) as tc:
        with (
            tc.tile_pool(name="big", bufs=1) as big,
            tc.tile_pool(name="acts", bufs=2) as acts,
            tc.tile_pool(name="ps1", bufs=4, space="PSUM") as ps1,
            tc.tile_pool(name="ps2", bufs=2, space="PSUM") as ps2,
            tc.tile_pool(name="psd", bufs=1, space="PSUM") as psd,
        ):
            # SBUF-resident weights for all 8 experts (128 KiB/partition).
            w1r = big.tile([128, EPC, 2, DC, H // 2], f8, name="w1r")
            w2r = big.tile([128, EPC, HC, D], f8, name="w2r")
            xtt = big.tile([128, EPC, DC, CAP], f8, name="xtt")
            b1t = big.tile([128, EPC, HC], f32, name="b1t")
            warm = big.tile([128, 512], f8, name="warm")
            warmf = big.tile([128, 1], f32, name="warmf")
            scr = big.tile([128, 1], f32, name="scr")

            # ---- DMA issue plan (per-engine queues drain in program order)
            # sync/HWDGE ring 1: expert-0 weights first (the DMA fabric is
            # nearly dead before ~10us and HWDGE starts ~3us before SWDGE),
            # w1 in quarters so mm1 can start on the first 0.25 MB; later
            # the y stores.
            nc.sync.dma_start(w1r[:, 0, 0], src_w1(0)[:, 0])
            nc.sync.dma_start(w2r[:, 0, :HC // 2], src_w2(0)[:, :HC // 2])
            nc.sync.dma_start(w1r[:, 0, 1], src_w1(0)[:, 1])
            nc.sync.dma_start(w2r[:, 0, HC // 2:], src_w2(0)[:, HC // 2:])
            # scalar/HWDGE ring 2: biases first (tiny, unblocks the relu
            # path), then tokens for experts 0-1, then the rest.
            nc.scalar.dma_start(b1t[:], b1[:])
            nc.scalar.dma_start(xtt[:, :2], xt[:, :2])
            nc.scalar.dma_start(xtt[:, 2:], xt[:, 2:])
            # gpsimd/SWDGE: experts 1..7 (the bulk, ~14 MB), in consumption
            # order.  The last expert is split finer so its compute can
            # chase the final bytes of the stream.
            for i in range(1, EPC):
                if i == EPC - 1:
                    nc.gpsimd.dma_start(w1r[:, i, 0], src_w1(i)[:, 0])
                    nc.gpsimd.dma_start(w2r[:, i, :HC // 2],
                                        src_w2(i)[:, :HC // 2])
                    nc.gpsimd.dma_start(w1r[:, i, 1], src_w1(i)[:, 1])
                    nc.gpsimd.dma_start(w2r[:, i, HC // 2:],
                                        src_w2(i)[:, HC // 2:])
                else:
                    nc.gpsimd.dma_start(w1r[:, i], src_w1(i))
                    nc.gpsimd.dma_start(w2r[:, i], src_w2(i))

            # ---- PE pre-warm: dummy matmuls on a zeroed tile keep the PE
            # busy from ~7us so the HAM clock-gate opens (2.4 GHz) right as
            # the first real matmul becomes ready (~11us).  They must run
            # back-to-back into the real stream -- any >1us PE idle resets
            # the HAM busy-window accumulation.
            nc.vector.memset(warm[:], 0.0)
            nc.vector.memset(warmf[:], 0.0)
            pd = psd.tile([64, 512], f32, name="pd")
            for _ in range(NWARM):
                nc.tensor.matmul(pd[:], warm[:, :64], warm[:],
                                 start=True, stop=True)
            # ACT-table preload (first activation pays a ~1.3us table load).
            # Reads only the locally-memset warmf -- it must NOT wait on any
            # DMA, or it blocks the whole relu queue behind it.
            nc.scalar.activation(scr[:], warmf[:], relu, bias=warmf[:])

            # ---- per-expert compute, chasing the weight stream
            for i in range(EPC):
                # h^T = relu(W1^T x^T + b1), produced [h, token] so mm2 can
                # contract over h on the partition dim.  The relu+bias (which
                # also casts to fp8 for DoubleRow mm2) alternates between
                # ScalarE and VectorE to halve the serial cost.
                # mm2 (DoubleRow fp8: 256-row contraction, 8 MMs of
                # N=512 instead of 16) is issued in two half-groups
                # interleaved with mm1, matching the weight-piece arrival
                # order w1a, w2a, w1b, w2b.
                ht = acts.tile([128, HC, CAP], f8, name="ht")
                p2 = ps2.tile([CAP, D], f32, name="p2")

                def mm1_chunk(j):
                    p1 = ps1.tile([128, CAP], f32, name="p1")
                    hh, jj = divmod(j, HC // 2)
                    for c in range(DC):
                        nc.tensor.matmul(
                            p1[:],
                            w1r[:, i, hh, c, jj * 128:(jj + 1) * 128],
                            xtt[:, i, c, :],
                            start=(c == 0),
                            stop=(c == DC - 1),
                        )
                    if j % 2 == 0:
                        nc.scalar.activation(ht[:, j, :], p1[:], relu,
                                             bias=b1t[:, i, j:j + 1])
                    else:
                        nc.vector.tensor_scalar(
                            ht[:, j, :], p1[:], b1t[:, i, j:j + 1], 0.0,
                            alu_add, alu_max)

                def dr_chunk(k):
                    nc.tensor.matmul(
                        p2[:],
                        ht[:, 2 * k:2 * k + 2, :],
                        w2r[:, i, 2 * k:2 * k + 2, :],
                        start=(k == 0),
                        stop=(k == HC // 2 - 1),
                        perf_mode=DR,
                        skip_group_check=True,
                    )

                for j in range(10):
                    mm1_chunk(j)
                for k in range(4):
                    dr_chunk(k)
                for j in range(10, HC):
                    mm1_chunk(j)
                for k in range(4, HC // 2):
                    dr_chunk(k)
                yt = acts.tile([CAP, D], bf, name="yt")
                nc.vector.tensor_copy(yt[:], p2[:])
                nc.sync.dma_start(y[i], yt[:])

    # Bacc lowering: splits excess per-instruction sem waits onto
    # InstEventSemaphore, moves matmul waits onto ldweights, inserts
    # activation table loads -- required for walrus codegen (1 wait slot
    # per 64B ISA instruction).
    nc.compile()
    return nc


def _get_nc():
    if "nc" not in _NC_CACHE:
        _NC_CACHE["nc"] = _build_bass()
    return _NC_CACHE["nc"]


def kernel(x, Wg, bg, W1, b1, W2, b2, gamma, beta):
    global LAST_RESULT
    x = np.asarray(x, np.float32)
    Wg = np.asarray(Wg, np.float32)
    bg = np.asarray(bg, np.float32)
    W1 = np.asarray(W1, np.float32)
    b1 = np.asarray(b1, np.float32)
    W2 = np.asarray(W2, np.float32)
    b2 = np.asarray(b2, np.float32)
    gamma = np.asarray(gamma, np.float32)
    beta = np.asarray(beta, np.float32)

    xf = x.reshape(T, D)

    # ---- gating: softmax over experts, top-2 (ties -> lower index, as top_k)
    logits = xf @ Wg + bg
    logits -= logits.max(-1, keepdims=True)
    probs = np.exp(logits)
    probs /= probs.sum(-1, keepdims=True)
    idx = np.argsort(-probs, axis=-1, kind="stable")[:, :TOPK]   # [T, K]
    vals = np.take_along_axis(probs, idx, axis=-1)               # [T, K]

    # ---- per-expert token lists (the all-to-all "sharding by routing")
    slot = np.full((T, TOPK), -1, np.int64)
    toks_per_e = []
    overflow = []  # (expert, token_ids) pairs beyond CAP -> host fallback
    for e in range(E):
        te = np.nonzero((idx == e).any(-1))[0]
        if len(te) > CAP:
            overflow.append((e, te[CAP:]))
            te = te[:CAP]
        toks_per_e.append(te)
        if len(te):
            k_of = (idx[te] == e).argmax(-1)
            slot[te, k_of] = np.arange(len(te))

    # ---- pack per-core device inputs (layouts match SBUF tiles exactly)
    xth = np.zeros((E, 128, DC, CAP), FP8)
    for e in range(E):
        te = toks_per_e[e]
        if len(te):
            blk = xf[te].T.reshape(DC, 128, len(te)).transpose(1, 0, 2)
            xth[e, :, :, :len(te)] = blk.astype(FP8)
    wq = lambda a: (a * WSCALE).astype(FP8)
    # [e, p, s, c, h']: H-halves (s) outermost, then the DC contraction
    # chunks -- matches the w1r SBUF tile so half-w1 DMAs are contiguous.
    w1h = wq(W1).reshape(E, DC, 128, 2, H // 2).transpose(0, 2, 3, 1, 4)
    w2h = wq(W2).reshape(E, HC, 128, D).transpose(0, 2, 1, 3)
    w12h = np.concatenate([w1h.reshape(E, 128, DC * H),
                           w2h.reshape(E, 128, HC * D)], axis=2)
    b1s = b1 * WSCALE
    b1h = np.ascontiguousarray(b1s.reshape(E, HC, 128).transpose(0, 2, 1))

    in_maps = []
    for c in range(NCORES):
        sl = slice(c * EPC, (c + 1) * EPC)
        in_maps.append({
            "xt": np.ascontiguousarray(xth[sl].transpose(1, 0, 2, 3)),
            "w12": w12h[sl],
            "b1": np.ascontiguousarray(b1h[sl].transpose(1, 0, 2)),
        })

    # ---- run on the 8 cores
    from concourse.bass_utils import run_bass_kernel_spmd
    nc = _get_nc()
    res = run_bass_kernel_spmd(nc, in_maps, list(range(NCORES)),
                               trace=PROFILE)
    LAST_RESULT = res
    y_all = np.concatenate([r["y"] for r in res.results],
                           axis=0).astype(np.float32)             # [E,CAP,D]
    y_all /= WSCALE * WSCALE

    # ---- unshard: scatter expert outputs back by routing, combine, LN
    ok = slot >= 0
    sl = np.where(ok, slot, 0)
    contrib = y_all[idx, sl] + b2[idx]                 # [T, K, D]
    out = xf + (vals[..., None] * contrib * ok[..., None]).sum(1)

    for e, te in overflow:  # practically never taken (CAP >> max count)
        k_of = (idx[te] == e).argmax(-1)
        w = vals[te, k_of]
        h = np.maximum(xf[te] @ W1[e] + b1[e], 0.0)
        out[te] += w[:, None] * (h @ W2[e] + b2[e])

    mu = out.mean(-1, keepdims=True)
    var = ((out - mu) ** 2).mean(-1, keepdims=True)
    o = (out - mu) / np.sqrt(var + EPS) * gamma + beta
    return o.reshape(B, S, D).astype(np.float32)


# revision 17
# speedup vs baseline: 1.4141x; 1.0090x over previous
"""Expert-parallel sparse MoE kernel for Trainium2 (8 NeuronCores).

Reference model: dense MoE (every expert on every token) followed by a
top-2-sparse combine, residual add, and LayerNorm.  Mathematically only the
top-2 experts per token contribute to the output, so the kernel routes each
token to its top-2 experts and only computes those expert FFNs.

Sharding: expert-parallel.  Each of the 8 cores owns 8 of the 64 experts and
receives the tokens routed to them (all-to-all by routing, done host-side as
part of sharding).  The device streams the expert weights (the dominant
memory traffic, fp8) into a fully SBUF-resident buffer -- all weight DMAs are
issued up-front with no buffer recycling, so the HBM stream runs gap-free at
line rate.  Compute chases the stream one expert at a time:
  mm1: h^T = relu(W1^T x^T + b1)   (w1 stationary fp8+FWL, x moving, N=96)
  mm2: y = h^T.T @ W2              (DoubleRow fp8: both operands fp8,
                                    256-row contraction, 8 MMs of N=512)
The host applies gate weights + b2 during the unshard/scatter, adds the
residual, and normalizes.
"""

import numpy as np
import ml_dtypes

B, S, D, H, E, TOPK = 2, 1024, 512, 2048, 64, 2
T = B * S
NCORES = 8
EPC = E // NCORES          # experts per core
CAP = 96                   # token capacity per expert (observed max 95;
                           # overflow tokens fall back to exact host compute)
DC = D // 128              # 4 contraction chunks for x @ W1
HC = H // 128              # 16 contraction chunks for h @ W2
EPS = 1e-5
BF16 = ml_dtypes.bfloat16
NWARM = 12                 # dummy N=512 matmuls that hold the PE HAM
                           # clock-gate open while the first weight DMA
                           # is in flight (~430ns each cold)

PROFILE = False            # set True (module-level) to capture an NTFF trace
LAST_RESULT = None         # BassKernelResults of the last run (for test.py)

# fp8 everywhere on-device: W1/W2 scaled by WSCALE host-side and stored e4m3;
# x and the hidden h are also e4m3 (their error is ~2-3% relative on the
# expert outputs, which enter the final LN-dominated output at ~0.03x -- far
# inside the 2e-2 absmax tolerance).  1/WSCALE^2 descale folds into the
# host-side combine.
WSCALE = 16.0
FP8 = ml_dtypes.float8_e4m3fn

_NC_CACHE = {}


def _build_bass():
    """Build the per-core Bass/Tile program (identical on all 8 cores)."""
    import concourse.bacc as bacc
    import concourse.mybir as mybir
    from concourse import tile

    nc = bacc.Bacc("TRN2", target_bir_lowering=False, debug=False,
                   num_devices=NCORES)

    bf = mybir.dt.bfloat16
    f32 = mybir.dt.float32
    f8 = mybir.dt.float8e4
    xt = nc.dram_tensor("xt", [128, EPC, DC, CAP], f8, kind="ExternalInput")
    # W1|W2 fused per expert: [d-part, DC*H (w1) + HC*D (w2)]
    w12 = nc.dram_tensor("w12", [EPC, 128, DC * H + HC * D], f8,
                         kind="ExternalInput")
    b1 = nc.dram_tensor("b1", [128, EPC, HC], f32, kind="ExternalInput")
    y = nc.dram_tensor("y", [EPC, CAP, D], bf, kind="ExternalOutput")

    relu = mybir.ActivationFunctionType.Relu
    DR = mybir.MatmulPerfMode.DoubleRow
    alu_add = mybir.AluOpType.add
    alu_max = mybir.AluOpType.max

    # w1 per-expert DRAM layout: [128, 2, DC, H/2] -- H-halves outermost so
    # a half-w1 DMA is per-partition contiguous (4 KiB descriptors; smaller
    # strided descriptors measured 2-8x slower on the HWDGE ring).
    src_w1 = lambda i: w12[i][:, :DC * H].rearrange(
        "p (s c h) -> p s c h", s=2, c=DC)
    src_w2 = lambda i: w12[i][:, DC * H:].rearrange("p (c d) -> p c d", c=HC)

    with tile.TileContext(nc) as tc:
        with (
            tc.tile_pool(name="big", bufs=1) as big,
            tc.tile_pool(name="acts", bufs=2) as acts,
            tc.tile_pool(name="ps1", bufs=4, space="PSUM") as ps1,
            tc.tile_pool(name="ps2", bufs=2, space="PSUM") as ps2,
            tc.tile_pool(name="psd", bufs=1, space="PSUM") as psd,
        ):
            # SBUF-resident weights for all 8 experts (128 KiB/partition).
            w1r = big.tile([128, EPC, 2, DC, H // 2], f8, name="w1r")
            w2r = big.tile([128, EPC, HC, D], f8, name="w2r")
            xtt = big.tile([128, EPC, DC, CAP], f8, name="xtt")
            b1t = big.tile([128, EPC, HC], f32, name="b1t")
            warm = big.tile([128, 512], f8, name="warm")
            warmf = big.tile([128, 1], f32, name="warmf")
            scr = big.tile([128, 1], f32, name="scr")

            # ---- DMA issue plan (per-engine queues drain in program order)
            # ALL weights ride the gpsimd/SWDGE queue in consumption order --
            # the HWDGE rings measured 3-10x slower than SWDGE during the
            # early fabric ramp, so expert 0 goes at the head of the SWDGE
            # queue (first MB lands ~12us).  First and last experts are
            # split w1a/w2a/w1b/w2b so compute chases the piece stream.
            # scalar/HWDGE ring: biases first (tiny, unblocks the relu
            # path), then the token tiles.  sync ring: y stores only.
            nc.scalar.dma_start(b1t[:], b1[:])
            nc.scalar.dma_start(xtt[:, :2], xt[:, :2])
            nc.scalar.dma_start(xtt[:, 2:], xt[:, 2:])
            for i in range(EPC):
                if i in (0, EPC - 1):
                    nc.gpsimd.dma_start(w1r[:, i, 0], src_w1(i)[:, 0])
                    nc.gpsimd.dma_start(w2r[:, i, :HC // 2],
                                        src_w2(i)[:, :HC // 2])
                    nc.gpsimd.dma_start(w1r[:, i, 1], src_w1(i)[:, 1])
                    nc.gpsimd.dma_start(w2r[:, i, HC // 2:],
                                        src_w2(i)[:, HC // 2:])
                else:
                    nc.gpsimd.dma_start(w1r[:, i], src_w1(i))
                    nc.gpsimd.dma_start(w2r[:, i], src_w2(i))

            # ---- PE pre-warm: dummy matmuls on a zeroed tile keep the PE
            # busy from ~7us so the HAM clock-gate opens (2.4 GHz) right as
            # the first real matmul becomes ready (~11us).  They must run
            # back-to-back into the real stream -- any >1us PE idle resets
            # the HAM busy-window accumulation.
            nc.vector.memset(warm[:], 0.0)
            nc.vector.memset(warmf[:], 0.0)
            pd = psd.tile([64, 512], f32, name="pd")
            for _ in range(NWARM):
                nc.tensor.matmul(pd[:], warm[:, :64], warm[:],
                                 start=True, stop=True)
            # ACT-table preload (first activation pays a ~1.3us table load).
            # Reads only the locally-memset warmf -- it must NOT wait on any
            # DMA, or it blocks the whole relu queue behind it.
            nc.scalar.activation(scr[:], warmf[:], relu, bias=warmf[:])

            # ---- per-expert compute, chasing the weight stream
            for i in range(EPC):
                # h^T = relu(W1^T x^T + b1), produced [h, token] so mm2 can
                # contract over h on the partition dim.  The relu+bias (which
                # also casts to fp8 for DoubleRow mm2) alternates between
                # ScalarE and VectorE to halve the serial cost.
                # mm2 (DoubleRow fp8: 256-row contraction, 8 MMs of
                # N=512 instead of 16) is issued in two half-groups
                # interleaved with mm1, matching the weight-piece arrival
                # order w1a, w2a, w1b, w2b.
                ht = acts.tile([128, HC, CAP], f8, name="ht")
                p2 = ps2.tile([CAP, D], f32, name="p2")

                def mm1_chunk(j):
                    p1 = ps1.tile([128, CAP], f32, name="p1")
                    hh, jj = divmod(j, HC // 2)
                    for c in range(DC):
                        nc.tensor.matmul(
                            p1[:],
                            w1r[:, i, hh, c, jj * 128:(jj + 1) * 128],
                            xtt[:, i, c, :],
                            start=(c == 0),
                            stop=(c == DC - 1),
                        )
                    if j % 2 == 0:
                        nc.scalar.activation(ht[:, j, :], p1[:], relu,
                                             bias=b1t[:, i, j:j + 1])
                    else:
                        nc.vector.tensor_scalar(
                            ht[:, j, :], p1[:], b1t[:, i, j:j + 1], 0.0,
                            alu_add, alu_max)

                def dr_chunk(k):
                    nc.tensor.matmul(
                        p2[:],
                        ht[:, 2 * k:2 * k + 2, :],
                        w2r[:, i, 2 * k:2 * k + 2, :],
                        start=(k == 0),
                        stop=(k == HC // 2 - 1),
                        perf_mode=DR,
                        skip_group_check=True,
                    )

                for j in range(10):
                    mm1_chunk(j)
                for k in range(4):
                    dr_chunk(k)
                for j in range(10, HC):
                    mm1_chunk(j)
                for k in range(4, HC // 2):
                    dr_chunk(k)
                yt = acts.tile([CAP, D], bf, name="yt")
                nc.vector.tensor_copy(yt[:], p2[:])
                nc.sync.dma_start(y[i], yt[:])

    # Bacc lowering: splits excess per-instruction sem waits onto
    # InstEventSemaphore, moves matmul waits onto ldweights, inserts
    # activation table loads -- required for walrus codegen (1 wait slot
    # per 64B ISA instruction).
    nc.compile()
    return nc


def _get_nc():
    if "nc" not in _NC_CACHE:
        _NC_CACHE["nc"] = _build_bass()
    return _NC_CACHE["nc"]


def kernel(x, Wg, bg, W1, b1, W2, b2, gamma, beta):
    global LAST_RESULT
    x = np.asarray(x, np.float32)
    Wg = np.asarray(Wg, np.float32)
    bg = np.asarray(bg, np.float32)
    W1 = np.asarray(W1, np.float32)
    b1 = np.asarray(b1, np.float32)
    W2 = np.asarray(W2, np.float32)
    b2 = np.asarray(b2, np.float32)
    gamma = np.asarray(gamma, np.float32)
    beta = np.asarray(beta, np.float32)

    xf = x.reshape(T, D)

    # ---- gating: softmax over experts, top-2 (ties -> lower index, as top_k)
    logits = xf @ Wg + bg
    logits -= logits.max(-1, keepdims=True)
    probs = np.exp(logits)
    probs /= probs.sum(-1, keepdims=True)
    idx = np.argsort(-probs, axis=-1, kind="stable")[:, :TOPK]   # [T, K]
    vals = np.take_along_axis(probs, idx, axis=-1)               # [T, K]

    # ---- per-expert token lists (the all-to-all "sharding by routing")
    slot = np.full((T, TOPK), -1, np.int64)
    toks_per_e = []
    overflow = []  # (expert, token_ids) pairs beyond CAP -> host fallback
    for e in range(E):
        te = np.nonzero((idx == e).any(-1))[0]
        if len(te) > CAP:
            overflow.append((e, te[CAP:]))
            te = te[:CAP]
        toks_per_e.append(te)
        if len(te):
            k_of = (idx[te] == e).argmax(-1)
            slot[te, k_of] = np.arange(len(te))

    # ---- pack per-core device inputs (layouts match SBUF tiles exactly)
    xth = np.zeros((E, 128, DC, CAP), FP8)
    for e in range(E):
        te = toks_per_e[e]
        if len(te):
            blk = xf[te].T.reshape(DC, 128, len(te)).transpose(1, 0, 2)
            xth[e, :, :, :len(te)] = blk.astype(FP8)
    wq = lambda a: (a * WSCALE).astype(FP8)
    # [e, p, s, c, h']: H-halves (s) outermost, then the DC contraction
    # chunks -- matches the w1r SBUF tile so half-w1 DMAs are contiguous.
    w1h = wq(W1).reshape(E, DC, 128, 2, H // 2).transpose(0, 2, 3, 1, 4)
    w2h = wq(W2).reshape(E, HC, 128, D).transpose(0, 2, 1, 3)
    w12h = np.concatenate([w1h.reshape(E, 128, DC * H),
                           w2h.reshape(E, 128, HC * D)], axis=2)
    b1s = b1 * WSCALE
    b1h = np.ascontiguousarray(b1s.reshape(E, HC, 128).transpose(0, 2, 1))

    in_maps = []
    for c in range(NCORES):
        sl = slice(c * EPC, (c + 1) * EPC)
        in_maps.append({
            "xt": np.ascontiguousarray(xth[sl].transpose(1, 0, 2, 3)),
            "w12": w12h[sl],
            "b1": np.ascontiguousarray(b1h[sl].transpose(1, 0, 2)),
        })

    # ---- run on the 8 cores
    from concourse.bass_utils import run_bass_kernel_spmd
    nc = _get_nc()
    res = run_bass_kernel_spmd(nc, in_maps, list(range(NCORES)),
                               trace=PROFILE)
    LAST_RESULT = res
    y_all = np.concatenate([r["y"] for r in res.results],
                           axis=0).astype(np.float32)             # [E,CAP,D]
    y_all /= WSCALE * WSCALE

    # ---- unshard: scatter expert outputs back by routing, combine, LN
    ok = slot >= 0
    sl = np.where(ok, slot, 0)
    contrib = y_all[idx, sl] + b2[idx]                 # [T, K, D]
    out = xf + (vals[..., None] * contrib * ok[..., None]).sum(1)

    for e, te in overflow:  # practically never taken (CAP >> max count)
        k_of = (idx[te] == e).argmax(-1)
        w = vals[te, k_of]
        h = np.maximum(xf[te] @ W1[e] + b1[e], 0.0)
        out[te] += w[:, None] * (h @ W2[e] + b2[e])

    mu = out.mean(-1, keepdims=True)
    var = ((out - mu) ** 2).mean(-1, keepdims=True)
    o = (out - mu) / np.sqrt(var + EPS) * gamma + beta
    return o.reshape(B, S, D).astype(np.float32)
